# revision 52
# baseline (speedup 1.0000x reference)
"""Trainium2 Bass kernel for nn_DistLayer (segment-mean pooling + fc + BatchNorm + ReLU).

Contract: kernel(**inputs) takes FULL unsharded numpy inputs and returns the
FULL [131072, 256] float32 output. Internally shards rows across 8 NeuronCores.

Math (reference):
    pooled_atom = segment_mean(x[:, :128], atom_idx)[atom_idx]
    pooled_ele  = segment_mean(x[:, 128:256], atom_idx)[ele_idx]
    h = concat([x_atom, pooled_atom, x_ele, pooled_ele, x_dist]) @ W1 + b1
    out = relu(batchnorm(h))                    (training-mode batch stats)

v4 structure (per core, h kept feature-major "h^T" [2 x 128, rows] in f16):
  P1   : rows cut into 64 global units of 2048 rows (8/core). Unit u owns segs
         [gstart[u], ulast[u]]. Per unit, one shared one-hot array (fp8) with
         AL lanes (own segs at lane seg-gstart, the previous unit's boundary
         seg at lane AL-1 via a PRE-halo tile; POST-halo completes the last
         owned seg) drives TWO interleaved matmul groups over 24 row-tiles:
           - ele pooling   : onehot^T @ x_ele      -> [AL, 128]   (PSUM)
           - atom pooling^T: x_atom^T-as-lhsT @ oh -> [128, AL]   (PSUM)
         Ele sums (lanes 0..SLS) are scaled by 1/count and flushed f16 to the
         core's ele slab. Transposed atom sums are multiplied by W1pa in
         SEGMENT space (qa^T [AL, 256] per unit), scaled, kept in SBUF fp8.
  AG   : AllGather ONLY the ele slabs -> gslab [8, 8*SLS, 128] f16. The atom
         side never leaves the core.
  A    : h_x^T = Wx^T x^T per block-pair (f16) -> PSUM [128,1024] -> one Act
         copy into persistent SBUF hsb f16.
  B    : per unit: four 512-idx dma_gathers pull pooled_ele rows from gslab;
         per block/half: hq = qa-expansion matmul (selection one-hot, fp8) +
         Wpe^T gat (f16) into a [128,1024] PSUM pair; ONE DVE add per pair
         into hsb emitting sum(h); per-unit Act Square pass emits sum(h^2).
  stats: AllGather per-core [128,4] sums (NOT AllReduce: 1.875x cheaper),
         reduce locally, build scale/bias.
  P3   : fused scale+bias+relu in [128, 2x1024] chunks streamed to DRAM out,
         alternating DMA queues.
"""

import os
from contextlib import ExitStack

import ml_dtypes
import numpy as np

import concourse.bass as bass
import concourse.tile as tile
from concourse import bacc, mybir
from concourse.bass_utils import run_bass_kernel_spmd

LAST_NC = None  # most recent built program (for cost-model timing in test.py)
SAFE = set(filter(None, os.environ.get("K_SAFE", "").split(",")))

F32 = mybir.dt.float32
F16 = mybir.dt.float16
F8 = mybir.dt.float8e4
I16 = mybir.dt.int16
NPF8 = ml_dtypes.float8_e4m3

N_AE = 128
NUM_SEG = 4096
EPS = 1e-5
D_IN = 384            # x feature dim
D_OUT = 256           # output feature dim
BLK = 512             # rows per block
TPB = BLK // 128      # row-tiles per block
GSZ = 4               # blocks per unit
UROWS = BLK * GSZ     # rows per unit (2048)
NGRP = 8              # units per core
NBLK = NGRP * GSZ     # blocks per core (32)
NPAIR = NBLK // 2     # block pairs per core (16)
RPC = NBLK * BLK      # rows per core (16384)
N_CORES = 8
N_TOTAL = N_CORES * RPC

# fp8 for the pooling inputs / one-hots / qa unless disabled
P_DT = F16 if "nof8" in SAFE else F8
NP_PDT = np.float16 if "nof8" in SAFE else NPF8


def _pad8(v):
    return (int(v) + 7) // 8 * 8


def _wrap_idx16(idx):
    """dma_gather index layout: idx i at [i%16, i//16], replicated to 128 partitions."""
    n = idx.shape[0]
    w = idx.reshape(n // 16, 16).T.astype(np.int16)   # [16, n/16]
    return np.tile(w, (8, 1))                          # [128, n/16]


def build_program(n_cores, SLS, AL):
    """Build the (core-uniform) bass program."""
    W = NGRP * SLS        # ele-slab rows per core
    XW = TPB * 2 * N_AE   # xae columns per block (1024)
    nc = bacc.Bacc("TRN2", target_bir_lowering=False, debug=False,
                   num_devices=n_cores)

    # ---- I/O tensors (per-core) ----
    d_xt = nc.dram_tensor("xt", [NBLK, 128, 3 * BLK], F16, kind="ExternalInput").ap()
    d_xae = nc.dram_tensor("xae", [NBLK + 2, 128, XW], P_DT,
                           kind="ExternalInput").ap()
    d_oh = nc.dram_tensor("oh", [NGRP, 128, 24 * AL], P_DT, kind="ExternalInput").ap()
    d_sel = nc.dram_tensor("sel", [AL, NBLK * BLK], P_DT, kind="ExternalInput").ap()
    d_gidxe = nc.dram_tensor("gidxe", [NGRP, 128, UROWS // 16], I16,
                             kind="ExternalInput").ap()
    d_wx = nc.dram_tensor("wx", [D_IN, D_OUT], F16, kind="ExternalInput").ap()
    d_wpa = nc.dram_tensor("wpa", [N_AE, D_OUT], F16, kind="ExternalInput").ap()
    d_wpe = nc.dram_tensor("wpe", [N_AE, D_OUT], F16, kind="ExternalInput").ap()
    d_scl = nc.dram_tensor("scl", [AL, NGRP], F32, kind="ExternalInput").ap()
    d_gb = nc.dram_tensor("gb", [128, 4], F32, kind="ExternalInput").ap()
    d_eye = nc.dram_tensor("eye", [128, 128], F16, kind="ExternalInput").ap()

    d_out = nc.dram_tensor("out", [D_OUT, RPC], F32, kind="ExternalOutput").ap()

    groups = [list(range(n_cores))]
    AF = mybir.ActivationFunctionType

    with tile.TileContext(nc) as tc, ExitStack() as ctx:
        const = ctx.enter_context(tc.tile_pool(name="const", bufs=1))
        store = ctx.enter_context(tc.tile_pool(name="store", bufs=1))
        strm = ctx.enter_context(tc.tile_pool(name="strm", bufs=3))
        ps1 = ctx.enter_context(tc.tile_pool(name="ps1", bufs=1, space="PSUM"))
        ps2 = ctx.enter_context(tc.tile_pool(name="ps2", bufs=1, space="PSUM"))
        dram = ctx.enter_context(tc.tile_pool(name="dram", bufs=1, space="DRAM"))

        # internal DRAM
        pslab = dram.tile([W, N_AE], F16)                      # AG input (ele)
        gslab = dram.tile([n_cores, W, N_AE], F16, addr_space="Shared")
        statin = dram.tile([128, 4], F32)
        statout = dram.tile([n_cores, 128, 4], F32, addr_space="Shared")

        # ---- batched front loads (sync queue, xae/oh interleaved) ----
        XQ = [9, 9, 9, 7]                      # blocks per xae quarter
        XO = [0, 9, 18, 27]
        xaeq = []
        ohq = []
        for q in range(4):
            t = const.tile([128, XQ[q] * XW], P_DT, name=f"xae{q}")
            nc.sync.dma_start(
                t[:].rearrange("p (b f) -> p b f", b=XQ[q]),
                d_xae[XO[q]:XO[q] + XQ[q]].rearrange("b p f -> p b f"))
            xaeq.append(t)
            # oh rides the Act HWDGE ring: its ring sem is independent of the
            # sync ring, so unit g only waits for the quarters it really needs
            t = const.tile([128, 2 * 24 * AL], P_DT, name=f"oh{q}")
            nc.scalar.dma_start(t[:].rearrange("p (g w) -> p g w", g=2),
                                d_oh[2 * q:2 * q + 2].rearrange("g p w -> p g w"))
            ohq.append(t)

        def xae_sl(bi, c0, c1):
            q = min(bi // 9, 3)
            return xaeq[q][:, (bi - XO[q]) * XW + c0:(bi - XO[q]) * XW + c1]

        # ---- small constants on the Pool SWDGE queue (no HWDGE contention),
        # most-urgent first: the flush scale gates unit 0's P1 flush ----
        scl = const.tile([AL, NGRP], F32)
        nc.gpsimd.dma_start(scl[:], d_scl[:])
        eye = const.tile([128, 128], F16)
        nc.gpsimd.dma_start(eye[:], d_eye[:])
        wpa = const.tile([128, D_OUT], F16)
        nc.gpsimd.dma_start(wpa[:], d_wpa[:])
        # not needed before phase A / B / stats: keep out of the front FIFO
        with tc.tile_wait_until(0.022):
            wpe = const.tile([128, D_OUT], F16)
            nc.gpsimd.dma_start(wpe[:], d_wpe[:])
            wxr = const.tile([128, 3 * D_OUT], F16)
            nc.gpsimd.dma_start(wxr[:].rearrange("p (c f) -> p c f", c=3),
                                d_wx.rearrange("(c p) f -> p c f", p=128))
            gb = const.tile([128, 4], F32)
            nc.gpsimd.dma_start(gb[:], d_gb[:])
        # selc / gsbe are loaded on the DVE queue AFTER the P1 loop (they are
        # only needed in phase B; loading early would steal DMA bandwidth)
        selc = const.tile([AL, NBLK * BLK], P_DT)
        gsbe = const.tile([128, NGRP * (UROWS // 16)], I16)
        qa_sb = const.tile([AL, NGRP * D_OUT], P_DT)

        # persistent h^T store: 2 chunks of [128, RPC] f16
        hsb = [store.tile([128, RPC], F16, name=f"hsb{m}", tag=f"hsb{m}")
               for m in range(2)]
        sums = store.tile([128, 2 * NPAIR], F32)    # sum(h) per pair per m
        NSQ = NGRP + 2                              # units 0-6 + unit 7's pairs
        sq = store.tile([128, 2 * NSQ], F32)        # sum(h^2) pieces per m

        # ---- P1: per-unit pooling -> scaled flush -> transpose -> qa ----
        # ONE matmul per 128-row tile (moving 256 = atom|ele) keeps the PE
        # sequencer (71 ns/inst decode) off the critical path. The transpose
        # and qa matmuls are software-pipelined one unit late so PE never
        # waits on the Act/DVE PSUM round-trips. All 8 units flush into one
        # persistent SBUF tile; ONE DMA ships the ele slab for the AllGather.
        ssbc = const.tile([AL, NGRP * 2 * N_AE], F16)

        sgts = {}

        def tr_mm(gq):
            # atom half [AL, 128] -> [128, AL] via PE transpose, then off to
            # SBUF on DVE (consumed by qps one unit later)
            tseg = ps1.tile([128, AL], F16, name="tseg", tag="tseg")
            nc.tensor.transpose(tseg[:],
                                ssbc[:, gq * 2 * N_AE:gq * 2 * N_AE + N_AE],
                                eye[0:AL, 0:AL])
            sgt = strm.tile([128, AL], F16, name="sgt", tag="sgt", bufs=2)
            nc.vector.tensor_scalar_add(sgt[:], tseg[:], 0.0)
            sgts[gq] = sgt

        def qa_mm(gq):
            qps = ps1.tile([AL, D_OUT], F32, name="qps", tag="qps")
            nc.tensor.matmul(qps[:], sgts[gq][:], wpa[:], start=True, stop=True)
            nc.vector.tensor_scalar_add(qa_sb[:, gq * D_OUT:(gq + 1) * D_OUT],
                                        qps[:], 0.0)

        for g in range(NGRP):
            seg = ps1.tile([AL, 2 * N_AE], F32, name="seg", tag="seg", bufs=2)
            for t in range(24):
                bi = 4 * g + t // 4
                c0 = (t % 4) * 2 * N_AE
                ohs = ohq[g // 2][:, ((g % 2) * 24 + t) * AL:
                                  ((g % 2) * 24 + t + 1) * AL]
                nc.tensor.matmul(seg[:], ohs, xae_sl(bi, c0, c0 + 2 * N_AE),
                                 start=(t == 0), stop=(t == 23))
            # scaled flush (1/count per lane): [AL, atom128|ele128] f16
            nc.scalar.activation(ssbc[:, g * 2 * N_AE:(g + 1) * 2 * N_AE],
                                 seg[:], AF.Identity,
                                 bias=0.0, scale=scl[:, g:g + 1])
            if g >= 1:
                tr_mm(g - 1)       # transpose of the previous unit's sums
            if g >= 2:
                qa_mm(g - 2)       # qa matmul two units back (sgt ready)
        tr_mm(NGRP - 1)
        qa_mm(NGRP - 2)
        qa_mm(NGRP - 1)
        # one batched slab DMA (ele halves of all units)
        nc.scalar.dma_start(
            pslab[:].rearrange("(g w) f -> w g f", g=NGRP),
            ssbc[0:SLS, :].rearrange("p (g c f) -> p g c f", g=NGRP, c=2)[:, :, 1])

        # phase-B-only loads, time-gated so their transfers land in the AG
        # window instead of delaying P1 (the Tile scheduler reorders freely)
        with tc.tile_wait_until(float(os.environ.get("K_SELW", "0.030"))):
            nc.gpsimd.dma_start(selc[:], d_sel[:])
            nc.gpsimd.dma_start(gsbe[:].rearrange("p (g w) -> p g w", g=NGRP),
                                d_gidxe.rearrange("g p w -> p g w"))

        # ---- AllGather the ele slabs ----
        nc.gpsimd.collective_compute(
            "AllGather", mybir.AluOpType.bypass, replica_groups=groups,
            ins=[pslab.opt()], outs=[gslab.opt()])

        # ---- phase A: h_x^T = Wx^T x^T per block-pair -> hsb (f16) ----
        # xtr loads are held back so their transfers queue BEHIND the P1
        # flushes in the DMA FIFO (the AllGather is gated on those flushes)
        XTW = float(os.environ.get("K_XTW", "0.031"))
        for p in range(NPAIR):
            xtr = strm.tile([128, 2 * 3 * BLK], F16, name="xtr", tag="xtr",
                            bufs=2)
            with tc.tile_wait_until(XTW, enable="noxtw" not in SAFE):
                nc.sync.dma_start(xtr[:].rearrange("p (b f) -> p b f", b=2),
                                  d_xt[2 * p:2 * p + 2].rearrange("b p f -> p b f"))
            for m in range(2):
                hp = ps2.tile([128, 2 * BLK], F32, name=f"hp{m}", tag=f"hp{m}")
                for j in range(2):
                    for k in range(3):
                        nc.tensor.matmul(hp[:, BLK * j:BLK * (j + 1)],
                                         wxr[:, D_OUT * k + 128 * m:
                                             D_OUT * k + 128 * (m + 1)],
                                         xtr[:, 3 * BLK * j + BLK * k:
                                             3 * BLK * j + BLK * (k + 1)],
                                         start=(k == 0), stop=(k == 2))
                nc.scalar.copy(hsb[m][:, 2 * BLK * p:2 * BLK * (p + 1)], hp[:])

        # ---- phase B: per-unit gathers + expansion/ele matmuls -> hsb ----
        gview = gslab[:].rearrange("s w f -> (s w) f")
        GW = 1024 if "g1024" in SAFE else 512   # indices per dma_gather
        for g in range(NGRP):
            gat = strm.tile([128, UROWS], F16, name="gat", tag="gat", bufs=2)
            for s in range(UROWS // GW):
                nc.gpsimd.dma_gather(
                    out_ap=gat[:, GW * s:GW * (s + 1)].rearrange(
                        "p (a n) -> p a n", a=1),
                    in_ap=gview,
                    idxs_ap=gsbe[:, (UROWS // 16) * g + (GW // 16) * s:
                                 (UROWS // 16) * g + (GW // 16) * (s + 1)],
                    num_idxs=GW, num_idxs_reg=GW,
                    elem_size=N_AE, elem_step=N_AE, transpose=True)
            for jp in range(GSZ // 2):           # block pairs within the unit
                p = 2 * g + jp                   # global pair index
                for m in range(2):
                    hq = ps2.tile([128, 2 * BLK], F32, name=f"hp{m}",
                                  tag=f"hp{m}")
                    for j in range(2):
                        b = GSZ * g + 2 * jp + j
                        nc.tensor.matmul(hq[:, BLK * j:BLK * (j + 1)],
                                         qa_sb[:, g * D_OUT + 128 * m:
                                               g * D_OUT + 128 * (m + 1)],
                                         selc[:, BLK * b:BLK * (b + 1)],
                                         start=True, stop=False)
                        nc.tensor.matmul(hq[:, BLK * j:BLK * (j + 1)],
                                         wpe[:, 128 * m:128 * (m + 1)],
                                         gat[:, BLK * (2 * jp + j):
                                             BLK * (2 * jp + j + 1)],
                                         start=False, stop=True)
                    nc.vector.scalar_tensor_tensor(
                        out=hsb[m][:, 2 * BLK * p:2 * BLK * (p + 1)],
                        in0=hsb[m][:, 2 * BLK * p:2 * BLK * (p + 1)],
                        scalar=0.0, in1=hq[:],
                        op0=mybir.AluOpType.add, op1=mybir.AluOpType.add,
                        accum_out=sums[:, NPAIR * m + p:NPAIR * m + p + 1])
                    if g == NGRP - 1:
                        # last unit: square per pair right after its add, so
                        # only one short op trails the final add
                        dmp = strm.tile([128, UROWS], F16, name="dmp",
                                        tag="dmp", bufs=2)
                        sc = NSQ * m + NGRP - 1 + jp
                        nc.scalar.activation(
                            dmp[:, 0:2 * BLK],
                            hsb[m][:, 2 * BLK * p:2 * BLK * (p + 1)],
                            AF.Square, accum_out=sq[:, sc:sc + 1])
            if g < NGRP - 1:
                for m in range(2):   # square pass over the finished unit
                    dmp = strm.tile([128, UROWS], F16, name="dmp", tag="dmp",
                                    bufs=2)
                    sc = NSQ * m + g
                    nc.scalar.activation(
                        dmp[:], hsb[m][:, UROWS * g:UROWS * (g + 1)],
                        AF.Square, accum_out=sq[:, sc:sc + 1])

        # ---- BN stats: reduce, AllGather (cheaper than AllReduce), affine ----
        s4 = const.tile([128, 4], F32)
        for m in range(2):
            nc.vector.reduce_sum(s4[:, m:m + 1],
                                 sums[:, NPAIR * m:NPAIR * (m + 1)],
                                 axis=mybir.AxisListType.X)
            nc.vector.reduce_sum(s4[:, 2 + m:3 + m],
                                 sq[:, NSQ * m:NSQ * m + NGRP + 1],
                                 axis=mybir.AxisListType.X)
        nc.sync.dma_start(statin[:], s4[:])
        s4g = const.tile([128, 4], F32)
        if "arstat" in SAFE:
            statout2 = dram.tile([128, 4], F32, addr_space="Shared")
            nc.gpsimd.collective_compute(
                "AllReduce", mybir.AluOpType.add, replica_groups=groups,
                ins=[statin.opt()], outs=[statout2.opt()])
            nc.sync.dma_start(s4g[:], statout2[:])
        else:
            nc.gpsimd.collective_compute(
                "AllGather", mybir.AluOpType.bypass, replica_groups=groups,
                ins=[statin.opt()], outs=[statout.opt()])
            s4g32 = const.tile([128, 4 * n_cores], F32)
            nc.sync.dma_start(s4g32[:].rearrange("p (c f) -> p c f", c=n_cores),
                              statout[:].rearrange("c p f -> p c f"))
            s4v = s4g32[:].rearrange("p (c f) -> p c f", c=n_cores)
            for j in range(4):
                nc.vector.reduce_sum(s4g[:, j:j + 1], s4v[:, :, j],
                                     axis=mybir.AxisListType.X)

        # affine build, kept on DVE except one fused Rsqrt(var+eps) on Act
        n_total = float(N_TOTAL)
        s4n = const.tile([128, 4], F32)
        nc.vector.tensor_scalar_mul(s4n[:], s4g[:], 1.0 / n_total)
        mu = s4n[:, 0:2]                    # E[h]
        mu2 = const.tile([128, 2], F32)
        nc.vector.tensor_tensor(out=mu2[:], in0=mu, in1=mu,
                                op=mybir.AluOpType.mult)
        var = const.tile([128, 2], F32)
        nc.vector.tensor_tensor(out=var[:], in0=s4n[:, 2:4], in1=mu2[:],
                                op=mybir.AluOpType.subtract)
        vare = const.tile([128, 2], F32)
        nc.vector.tensor_scalar_add(vare[:], var[:], EPS)
        std = const.tile([128, 2], F32)
        nc.scalar.activation(std[:], vare[:], AF.Sqrt, bias=0.0)
        rstd = const.tile([128, 2], F32)
        nc.vector.reciprocal(rstd[:], std[:])
        a_t = const.tile([128, 2], F32)
        nc.vector.tensor_tensor(out=a_t[:], in0=gb[:, 0:2], in1=rstd[:],
                                op=mybir.AluOpType.mult)
        mua = const.tile([128, 2], F32)
        nc.vector.tensor_tensor(out=mua[:], in0=mu, in1=a_t[:],
                                op=mybir.AluOpType.mult)
        baff = const.tile([128, 2], F32)
        nc.vector.tensor_tensor(out=baff[:], in0=gb[:, 2:4], in1=mua[:],
                                op=mybir.AluOpType.subtract)

        # ---- P3: out = relu(h * a + b) in [128, 2x1024] chunks ----
        CW = 1024
        d_out_v = d_out.rearrange("(c p) n -> p c n", p=128)
        for cch in range(RPC // CW):
            osb = strm.tile([128, 2 * CW], F32, name="osb", tag="osb", bufs=3)
            for m in range(2):
                nc.scalar.activation(osb[:, CW * m:CW * (m + 1)],
                                     hsb[m][:, CW * cch:CW * (cch + 1)],
                                     AF.Relu,
                                     scale=a_t[:, m:m + 1], bias=baff[:, m:m + 1])
            eng = nc.sync if cch % 2 == 0 else nc.gpsimd
            eng.dma_start(
                d_out_v[:, :, CW * cch:CW * (cch + 1)],
                osb[:].rearrange("p (c n) -> p c n", c=2))

    nc.compile()
    return nc


def prep(x, atom_idx, ele_idx, W1, b1, gamma, beta, n_cores=N_CORES, build=True):
    x = np.asarray(x, dtype=np.float32)
    atom_idx = np.asarray(atom_idx).astype(np.int64)
    ele_idx = np.asarray(ele_idx).astype(np.int64)
    W1 = np.asarray(W1, dtype=np.float32)
    gamma = np.asarray(gamma, dtype=np.float32)
    beta = np.asarray(beta, dtype=np.float32)

    n = x.shape[0]
    assert n == N_TOTAL and n_cores == N_CORES
    assert np.all(np.diff(atom_idx) >= 0), "atom_idx must be sorted"

    counts = np.bincount(atom_idx, minlength=NUM_SEG).astype(np.int64)
    inv_cnt = (1.0 / np.maximum(counts, 1)).astype(np.float32)
    assert counts.max() <= BLK, "segment larger than halo window"

    nunit = n_cores * NGRP
    ufirst = atom_idx[np.arange(nunit) * UROWS]
    ulast = atom_idx[np.arange(1, nunit + 1) * UROWS - 1]
    gstart = np.empty(nunit, dtype=np.int64)
    gstart[0] = 0
    for u in range(1, nunit):
        gstart[u] = max(ufirst[u], ulast[u - 1] + 1)
    spans = ulast - gstart + 1
    assert spans.min() >= 1, "a unit owns no segment"
    SLS = _pad8(int(spans.max()) + 1)
    AL = _pad8(SLS + 1)
    assert AL <= 128, f"unit owned span too large: {spans.max()}"
    # forward halo containment: a unit's last seg spills <= BLK rows into next
    for u in range(nunit - 1):
        r = UROWS * (u + 1) + BLK
        if r < n:
            assert atom_idx[r] > ulast[u], "segment spans past halo window"

    # boundary seg (prev unit's last) per unit; its rows must start within the
    # 512-row pre-halo window
    bseg = np.empty(nunit, dtype=np.int64)
    bseg[0] = -1
    bseg[1:] = ulast[:-1]
    first_row = np.searchsorted(atom_idx, np.maximum(bseg, 0), side="left")
    for u in range(1, nunit):
        assert first_row[u] >= UROWS * u - BLK, "boundary seg exceeds pre-halo"

    # seg -> global ele-slab row
    W = NGRP * SLS
    owner = np.full(NUM_SEG, -1, dtype=np.int64)
    for u in range(nunit - 1, -1, -1):
        owner[gstart[u]:ulast[u] + 1] = u
    slabrow = np.full(NUM_SEG, SLS - 1, dtype=np.int64)   # empty segs -> zero row
    m_ = owner >= 0
    su = owner[m_]
    slabrow[m_] = (su // NGRP) * W + (su % NGRP) * SLS + \
        (np.arange(NUM_SEG)[m_] - gstart[su])

    # per-row unit-local lane (for sel): own segs at lane seg-gstart,
    # boundary seg (prev unit's last) at lane AL-1
    u_of_row = np.arange(n) // UROWS
    lane_row = np.where(atom_idx == bseg[u_of_row], AL - 1,
                        atom_idx - gstart[u_of_row])
    assert lane_row.min() >= 0 and lane_row.max() < AL

    in_maps = []
    for c in range(n_cores):
        r0 = RPC * c
        im = {}

        # pre-tiled transposed x (f16): [block, part(feat%128), chunk*512+row]
        xs = x[r0:r0 + RPC]
        im["xt"] = np.ascontiguousarray(
            xs.reshape(NBLK, BLK, 3, 128).transpose(0, 3, 2, 1)
        ).reshape(NBLK, 128, 3 * BLK).astype(np.float16)

        # x_ae row-major tiles, blocks r0/BLK - 1 .. r0/BLK + NBLK (incl halos)
        xae = np.zeros((NBLK + 2, 128, TPB * 2 * N_AE), dtype=NP_PDT)
        lo, hi = r0 - BLK, r0 + RPC + BLK
        clo, chi = max(lo, 0), min(hi, n)
        buf = np.zeros(((NBLK + 2) * BLK, 2 * N_AE), dtype=np.float32)
        buf[clo - lo:chi - lo] = x[clo:chi, :2 * N_AE]
        xae[:] = buf.astype(NP_PDT).reshape(
            NBLK + 2, TPB, 128, 2 * N_AE).transpose(0, 2, 1, 3).reshape(
            NBLK + 2, 128, TPB * 2 * N_AE)
        im["xae"] = xae

        # shared one-hot per unit [128, 24*AL]
        oh = np.zeros((NGRP, 128, 24 * AL), dtype=NP_PDT)
        for g in range(NGRP):
            u = NGRP * c + g
            glo = UROWS * u - BLK
            rows = np.arange(glo, glo + 24 * 128)
            ok = (rows >= 0) & (rows < n)
            rr = rows[ok]
            seg = atom_idx[rr]
            lane = np.where(seg == bseg[u], AL - 1,
                            np.where((seg >= gstart[u]) & (seg <= ulast[u]),
                                     seg - gstart[u], -1))
            sel_ok = lane >= 0
            ri = rr[sel_ok] - glo
            oh[g, ri % 128, (ri // 128) * AL + lane[sel_ok]] = 1.0
        im["oh"] = oh

        # selection one-hot for expansion [AL, NBLK*BLK]
        sel = np.zeros((AL, NBLK * BLK), dtype=NP_PDT)
        sel[lane_row[r0:r0 + RPC], np.arange(RPC)] = 1.0
        im["sel"] = sel

        # gather indices (ele), per unit
        gidxe = np.zeros((NGRP, 128, UROWS // 16), dtype=np.int16)
        for g in range(NGRP):
            rows = slabrow[ele_idx[r0 + UROWS * g:r0 + UROWS * (g + 1)]]
            gidxe[g] = _wrap_idx16(rows)
        im["gidxe"] = gidxe

        # weights / scales
        im["wx"] = np.ascontiguousarray(
            np.concatenate([W1[0:128], W1[256:384], W1[512:640]], axis=0)
        ).astype(np.float16)
        im["wpa"] = W1[128:256].astype(np.float16)
        im["wpe"] = W1[384:512].astype(np.float16)
        scl = np.zeros((AL, NGRP), dtype=np.float32)
        lanes_a = np.arange(AL)
        for g in range(NGRP):
            u = NGRP * c + g
            scl[:, g] = inv_cnt[np.minimum(gstart[u] + lanes_a, NUM_SEG - 1)]
            scl[AL - 1, g] = inv_cnt[max(bseg[u], 0)]
        im["scl"] = scl
        im["eye"] = np.eye(128, dtype=np.float16)
        gbt = np.zeros((128, 4), dtype=np.float32)
        gbt[:, 0:2] = gamma.reshape(2, 128).T
        gbt[:, 2:4] = beta.reshape(2, 128).T
        im["gb"] = gbt
        in_maps.append(im)

    nc = build_program(n_cores, SLS, AL) if build else None
    global LAST_NC
    LAST_NC = nc
    return nc, in_maps


def run(x, atom_idx, ele_idx, W1, b1, gamma, beta, n_cores=N_CORES, runner=None):
    nc, in_maps = prep(x, atom_idx, ele_idx, W1, b1, gamma, beta, n_cores)
    if runner is None:
        res = run_bass_kernel_spmd(nc, in_maps, core_ids=list(range(n_cores)))
        outs = [res.results[c]["out"] for c in range(n_cores)]
    else:
        outs = runner(nc, in_maps)

    full = np.concatenate(outs, axis=1)          # [256, n]
    return np.ascontiguousarray(full.T)          # [n, 256]


def kernel(**inputs):
    return run(inputs["x"], inputs["atom_idx"], inputs["ele_idx"],
               inputs["W1"], inputs["b1"], inputs["gamma"], inputs["beta"])
